# revision 6
# baseline (speedup 1.0000x reference)
"""Trainium2 Bass kernel for a causal self-attention transformer block.

Reference computation (per batch b):
    qkv = x @ w_qkv.T ; split into q, k, v heads (16 heads, dim 64)
    s   = (q @ k.T) * dh**-0.5, causal + padding mask
    a   = softmax(s, axis=j)
    o   = (a @ v) @ w_out.T + b_out ; out = o * m[:, None]

Sharding: pure data parallel — batch (8) across the 8 NeuronCores, weights
replicated. No collectives.

Per-core device program (v2 — engine-balanced attention loop):
  - inputs host-pre-transposed so every matmul contraction dim needs no
    on-chip transpose; matmul operands fp16 (1 cyc/row), fp32 PSUM accum.
  - per-d-tile xt/wv tiles so the first V-proj matmul waits only on the
    first DMA (not all 16).
  - attention per head-pair: scores computed transposed S_T[j,i], the two
    heads co-streaming as 64-row PE tiles into one 2-bank PSUM tile;
    ONE exp instruction covers both heads; chunks are aligned to the
    i=512 boundary so only 2 A@V accumulators are live at a time.
  - A@V emission lags the score/exp stream by 2 chunks so the exp (ACT)
    and the causal tri-mask (GPSIMD) finish off the PE critical path.
  - denominator via the mask column of v_aug (row 64 of the A@V psum);
    reciprocal_approx_fast; normalization of pair g-1 during pair g via
    K=2 sel2-matmul broadcast + DVE multiply.
  - next pair's q/k projection woven into the attention stream in
    single-matmul steps (3 pull slots per chunk, 36 yields per pair).
  - PSUM budget: scores 2x[128,2,512] (4 banks) + A@V 3x[128,512]
    (3 banks) + proj 1x[128,512] (1 bank) = 8 banks.
"""

import os
import numpy as np
from contextlib import ExitStack

import ml_dtypes
from concourse import bacc
import concourse.mybir as mybir
import concourse.tile as tile
from concourse.bass_utils import run_bass_kernel_spmd

D = 1024          # model dim
T = 1024          # sequence length
H = 16            # heads
DH = 64           # head dim
P = 128           # partitions
ND = D // P       # d-tiles
NT = T // P       # t-tiles
NPAIR = H // 2    # head pairs
SCALE = DH ** -0.5
F32 = mybir.dt.float32
F32R = mybir.dt.float32r
BF16 = mybir.dt.bfloat16
MULT = mybir.AluOpType.mult
EXP = mybir.ActivationFunctionType.Exp

_MM_MODE = os.environ.get("TRN_MM_DT", "fp16")
MM_DT = {"fp16": mybir.dt.float16, "bf16": BF16, "f32r": F32R}[_MM_MODE]
NP_MM = {"fp16": np.float16, "bf16": ml_dtypes.bfloat16,
         "f32r": np.float32}[_MM_MODE]

_CACHE = {}
LAST_RESULTS = None


def _chunks():
    """Attention chunk schedule: (ci, J, lo, w) with i-chunks aligned to the
    512 boundary so that only the current i-half's A@V accumulators are
    live.  ci=0 covers i in [J*128, 512) for J<4; ci=1 covers
    i in [max(512, J*128), 1024) for all J."""
    out = []
    for J in range(4):
        out.append((0, J, J * P, 512 - J * P))
    for J in range(NT):
        lo = max(512, J * P)
        out.append((1, J, lo, min(512, T - lo)))
    return out


def _emit(nc, tc, xT_d, wqk_d, wv_d, wo_d, bo_d, mcol_d, tri_d, ones_d,
          sel2_d, out_d):
    ctx = ExitStack()
    with ctx:
        const = ctx.enter_context(tc.tile_pool(name="const", bufs=1))
        xt_p = ctx.enter_context(tc.tile_pool(name="xt", bufs=1))
        wv_p = ctx.enter_context(tc.tile_pool(name="wv", bufs=1))
        vaug_p = ctx.enter_context(tc.tile_pool(name="vaug", bufs=1))
        qkT_p = ctx.enter_context(tc.tile_pool(name="qkT", bufs=2))
        wqk_p = ctx.enter_context(tc.tile_pool(name="wqk", bufs=2))
        pt_p = ctx.enter_context(tc.tile_pool(name="pt", bufs=1))
        oT_p = ctx.enter_context(tc.tile_pool(name="oT", bufs=1))
        wo_p = ctx.enter_context(tc.tile_pool(name="wo", bufs=1))
        osb_p = ctx.enter_context(tc.tile_pool(name="osb", bufs=4))
        den_p = ctx.enter_context(tc.tile_pool(name="den", bufs=2))
        psS = ctx.enter_context(tc.tile_pool(name="psS", bufs=2, space="PSUM"))
        psAV = ctx.enter_context(tc.tile_pool(name="psAV", bufs=3,
                                              space="PSUM"))
        psP = ctx.enter_context(tc.tile_pool(name="psP", bufs=1, space="PSUM"))

        # per-d-tile resident xT and wv tiles, DMA'd interleaved so the
        # V projection starts after the first pair of transfers.
        xT_r = xT_d.ap().rearrange("(n p) t -> n p t", p=P)
        wv_r = wv_d.ap().rearrange("(n p) t -> n p t", p=P)
        xts, wvts = [], []
        for q in range(ND):
            xt = xt_p.tile([P, T], MM_DT, tag=f"xt{q}", name=f"xt{q}")
            nc.sync.dma_start(out=xt[:], in_=xT_r[q])
            xts.append(xt)
            wvt = wv_p.tile([P, T], MM_DT, tag=f"wv{q}", name=f"wv{q}")
            nc.sync.dma_start(out=wvt[:], in_=wv_r[q])
            wvts.append(wvt)
            if q == 1:
                # constants issued behind the first two d-tiles
                mcol = const.tile([P, NT], F32, tag="mcol", name="mcol")
                nc.sync.dma_start(out=mcol[:], in_=mcol_d.ap())
                tri2 = const.tile([P, 2, P], MM_DT, tag="tri", name="tri2")
                nc.sync.dma_start(out=tri2[:], in_=tri_d.ap())
                ones = const.tile([1, P], F32R, tag="ones", name="ones")
                nc.sync.dma_start(out=ones[:], in_=ones_d.ap())
                sel2 = const.tile([2, P], F32R, tag="sel2", name="sel2")
                nc.sync.dma_start(out=sel2[:], in_=sel2_d.ap())
                bos = const.tile([1, D], F32R, tag="bos", name="bos")
                nc.sync.dma_start(out=bos[:], in_=bo_d.ap())

        # v_aug tiles [128 t, 16 h, 65]: per-head v columns * mask + mask col
        vaug = [
            vaug_p.tile([P, H, DH + 1], MM_DT, tag=f"va{t}", name=f"va{t}")
            for t in range(NT)
        ]

        # ---- Phase 1: V projection (natural layout). Two t-tiles per
        # group, each in one 2-bank psS tile (halves = 512-col chunks).
        for g2 in range(0, NT, 2):
            accs = [
                psS.tile([P, 2, 512], F32, tag="s", name=f"vps{g2}_{i}")
                for i in range(2)
            ]
            for d in range(ND):
                for i in range(2):
                    tt = g2 + i
                    for c in range(2):
                        nc.tensor.matmul(
                            accs[i][:, c, :],
                            xts[d][:, tt * P:(tt + 1) * P],
                            wvts[d][:, c * 512:(c + 1) * 512],
                            start=(d == 0),
                            stop=(d == ND - 1),
                        )
            for i in range(2):
                tt = g2 + i
                for c in range(2):
                    ps3 = accs[i][:, c, :].rearrange("p (h e) -> p h e", e=DH)
                    nc.vector.tensor_scalar(
                        vaug[tt][:, c * 8:(c + 1) * 8, 0:DH],
                        ps3,
                        mcol[:, tt:tt + 1],
                        None,
                        MULT,
                    )
        for tt in range(NT):
            nc.vector.tensor_copy(
                out=vaug[tt][:, :, DH],
                in_=mcol[:, tt:tt + 1].to_broadcast([P, H]),
            )

        # ---- Phase 2: per head-pair q/k projection + attention.
        def _normalize(oT, rcpg):
            bc = psS.tile([P, 2, 512], F32, tag="s", name="bc")
            for c in range(2):
                nc.tensor.matmul(
                    bc[:, c, :],
                    sel2[:],
                    rcpg[0:2, c * 512:(c + 1) * 512],
                    start=True, stop=True,
                )
                nc.vector.tensor_tensor(
                    oT[:, c * 512:(c + 1) * 512],
                    oT[:, c * 512:(c + 1) * 512],
                    bc[:, c, :],
                    MULT,
                )

        def _proj(g, qT, kT):
            """Generator emitting pair g's q/k projection in single-matmul
            steps (one PSUM bank, half-at-a-time)."""
            for dest, et in ((qT, g), (kT, NPAIR + g)):
                wt = wqk_p.tile([P, ND, P], MM_DT, tag="wqk", name="wqkt")
                nc.sync.dma_start(
                    out=wt[:],
                    in_=wqk_d.ap()[et].rearrange("n p e -> p n e"),
                )
                for half in range(2):
                    pp = psP.tile([P, 512], F32, tag="p", name="pp")
                    for d in range(ND):
                        nc.tensor.matmul(
                            pp[:], wt[:, d, :],
                            xts[d][:, half * 512:(half + 1) * 512],
                            start=(d == 0), stop=(d == ND - 1),
                        )
                        yield
                    nc.vector.tensor_copy(
                        out=dest[:, half * 512:(half + 1) * 512], in_=pp[:]
                    )
                    yield

        def _pull(it, n):
            if it is None:
                return
            for _ in range(n):
                try:
                    next(it)
                except StopIteration:
                    return

        oTs = []
        pending = None
        qkTs = {0: (
            qkT_p.tile([P, T], MM_DT, tag="qT", name="qT0"),
            qkT_p.tile([P, T], MM_DT, tag="kT", name="kT0"),
        )}
        _pull(_proj(0, *qkTs[0]), 999)

        # output-projection weights, issued while V-proj computes
        wo_all = wo_p.tile([P, NPAIR, T], MM_DT, tag="wo", name="wot")
        wo_r = wo_d.ap().rearrange("(n p) t -> p n t", p=P)
        for q in range(4):
            nc.sync.dma_start(
                out=wo_all[:, 2 * q:2 * q + 2, :],
                in_=wo_r[:, 2 * q:2 * q + 2, :],
            )
        wots = [wo_all[:, g, :] for g in range(NPAIR)]

        chunks = _chunks()
        LAG = 2

        for g in range(NPAIR):
            qT, kT = qkTs[g]
            if g + 1 < NPAIR:
                qkTs[g + 1] = (
                    qkT_p.tile([P, T], MM_DT, tag="qT", name=f"qT{g + 1}"),
                    qkT_p.tile([P, T], MM_DT, tag="kT", name=f"kT{g + 1}"),
                )
                nxt = _proj(g + 1, *qkTs[g + 1])
            else:
                nxt = None

            oT = oT_p.tile([P, T], MM_DT, tag=f"oT{g}", name=f"oT{g}")
            oTs.append(oT)
            deng = den_p.tile([1, 2, T], F32, tag="den", name=f"den{g}")
            den2 = den_p.tile([2, T], F32, tag="den2", name=f"den2_{g}")
            rf32 = den_p.tile([2, T], F32, tag="rf32", name=f"rf32_{g}")
            rcpg = den_p.tile([2, T], F32R, tag="rcp", name=f"rcp{g}")

            pt = [
                pt_p.tile([P, 2, T], MM_DT, tag=f"pt{J}", name=f"pt{J}")
                for J in range(NT)
            ]
            avs = {}

            def _av_emit(ci, J, lo, w):
                # A@V for one chunk (+ denominator row via the mask column)
                for h in range(2):
                    key = (h, ci)
                    if key not in avs:
                        avs[key] = psAV.tile(
                            [P, 512], F32, tag="av", name=f"av{h}{ci}"
                        )
                    co = lo - ci * 512
                    nc.tensor.matmul(
                        avs[key][0:DH + 1, co:co + w],
                        vaug[J][:, 2 * g + h, :],
                        pt[J][:, h, lo:lo + w],
                        start=(J == 0),
                        stop=(J == (3 if ci == 0 else NT - 1)),
                    )
                if J == (3 if ci == 0 else NT - 1):
                    for h in range(2):
                        av = avs.pop((h, ci))
                        hs = slice(h * DH, (h + 1) * DH)
                        cs = slice(ci * 512, ci * 512 + 512)
                        nc.scalar.copy(
                            out=oT[hs, cs], in_=av[0:DH, :]
                        )
                        nc.vector.tensor_copy(
                            out=deng[0:1, h, cs], in_=av[DH:DH + 1, :]
                        )

            lagq = []
            for (ci, J, lo, w) in chunks:
                sp = psS.tile([P, 2, 512], F32, tag="s", name="sps")
                for h in range(2):
                    hs = slice(h * DH, (h + 1) * DH)
                    nc.tensor.matmul(
                        sp[0:P, h, 0:w],
                        kT[hs, J * P:(J + 1) * P],
                        qT[hs, lo:lo + w],
                        start=True, stop=True,
                    )
                _pull(nxt, 1)
                nc.scalar.activation(
                    out=pt[J][:, :, lo:lo + w], in_=sp[:, :, 0:w],
                    func=EXP, scale=SCALE,
                )
                _pull(nxt, 1)
                if lo == J * P:
                    # causal mask on the diagonal 128-col block
                    nc.vector.tensor_tensor(
                        pt[J][:, :, lo:lo + P],
                        pt[J][:, :, lo:lo + P],
                        tri2[:],
                        MULT,
                    )
                lagq.append((ci, J, lo, w))
                if len(lagq) > LAG:
                    _av_emit(*lagq.pop(0))
                _pull(nxt, 1)
            for item in lagq:
                _av_emit(*item)
            _pull(nxt, 999)

            # reciprocal of the pair's denominators (off the PE critical path)
            nc.sync.dma_start(out=den2[:], in_=deng[:])
            nc.vector.reciprocal_approx_fast(out=rf32[:], in_=den2[:])
            with nc.allow_low_precision(reason="fp32r recip feeds matmul"):
                nc.vector.tensor_copy(out=rcpg[:], in_=rf32[:])

            # normalize the PREVIOUS pair now: its reciprocal has been ready
            # for a whole pair-iteration, so the PE never waits on it.
            if pending is not None:
                _normalize(*pending)
            pending = (oT, rcpg)

        # ---- Phase 3: output projection, accumulate over head-pair tiles,
        # bias via K=1 ones-matmul, then mask-multiply and store.
        _normalize(*pending)
        for tt in range(NT):
            acc = psS.tile([P, 2, 512], F32, tag="s", name=f"ops{tt}")
            for gg in range(NPAIR):
                for c in range(2):
                    nc.tensor.matmul(
                        acc[:, c, :],
                        oTs[gg][:, tt * P:(tt + 1) * P],
                        wots[gg][:, c * 512:(c + 1) * 512],
                        start=(gg == 0), stop=False,
                    )
            for c in range(2):
                nc.tensor.matmul(
                    acc[:, c, :],
                    ones[0:1, 0:P],
                    bos[0:1, c * 512:(c + 1) * 512],
                    start=False, stop=True,
                )
                osb = osb_p.tile([P, 512], F32, tag="osb", name="osb")
                nc.vector.tensor_scalar(
                    osb[:], acc[:, c, :], mcol[:, tt:tt + 1], None, MULT,
                )
                nc.sync.dma_start(
                    out=out_d.ap()[tt * P:(tt + 1) * P,
                                   c * 512:(c + 1) * 512],
                    in_=osb[:],
                )


def build_nc():
    nc = bacc.Bacc("TRN2", target_bir_lowering=False, debug=False,
                   num_devices=8)
    xT_d = nc.dram_tensor("xT", [D, T], MM_DT, kind="ExternalInput")
    wqk_d = nc.dram_tensor("wqk", [H, ND, P, P], MM_DT, kind="ExternalInput")
    wv_d = nc.dram_tensor("wv", [D, D], MM_DT, kind="ExternalInput")
    wo_d = nc.dram_tensor("wo", [D, D], MM_DT, kind="ExternalInput")
    bo_d = nc.dram_tensor("bo", [1, D], F32R, kind="ExternalInput")
    mcol_d = nc.dram_tensor("mcol", [P, NT], F32, kind="ExternalInput")
    tri_d = nc.dram_tensor("tri", [P, 2, P], MM_DT, kind="ExternalInput")
    ones_d = nc.dram_tensor("ones", [1, P], F32R, kind="ExternalInput")
    sel2_d = nc.dram_tensor("sel2", [2, P], F32R, kind="ExternalInput")
    out_d = nc.dram_tensor("out", [T, D], F32, kind="ExternalOutput")
    with tile.TileContext(nc) as tc:
        _emit(nc, tc, xT_d, wqk_d, wv_d, wo_d, bo_d, mcol_d, tri_d, ones_d,
              sel2_d, out_d)
    nc.compile()
    return nc


def _prep_shared(w_qkv, w_out, b_out):
    wqkT = np.ascontiguousarray(w_qkv[:2 * D].T)             # [d, e]
    wqk_tiles = np.ascontiguousarray(
        wqkT.reshape(ND, P, H, P).transpose(2, 0, 1, 3)
    ).astype(NP_MM)                                          # [16, 8, 128, 128]
    wv = np.ascontiguousarray(w_qkv[2 * D:].T).astype(NP_MM)  # [d, ev]
    wo = np.ascontiguousarray(w_out.T).astype(NP_MM)          # [d', e]
    bo = np.ascontiguousarray(b_out.reshape(1, D))
    tri1 = np.triu(np.ones((P, P), dtype=np.float32))
    tri = np.ascontiguousarray(
        np.stack([tri1, tri1], axis=1)
    ).astype(NP_MM)                                          # [128, 2, 128]
    ones = np.ones((1, P), dtype=np.float32)
    sel2 = np.zeros((2, P), dtype=np.float32)
    sel2[0, 0:DH] = 1.0
    sel2[1, DH:P] = 1.0
    return wqk_tiles, wv, wo, bo, tri, ones, sel2


def kernel(x, m, w_qkv, w_out, b_out, l=None, **_unused):
    global LAST_RESULTS
    x = np.asarray(x, dtype=np.float32)
    m = np.asarray(m, dtype=np.float32)
    w_qkv = np.asarray(w_qkv, dtype=np.float32)
    w_out = np.asarray(w_out, dtype=np.float32)
    b_out = np.asarray(b_out, dtype=np.float32)

    if "nc" not in _CACHE:
        _CACHE["nc"] = build_nc()
    nc = _CACHE["nc"]

    wqk_tiles, wv, wo, bo, tri, ones, sel2 = _prep_shared(w_qkv, w_out, b_out)
    in_maps = []
    for b in range(8):
        in_maps.append({
            "xT": np.ascontiguousarray(x[b].T).astype(NP_MM),
            "wqk": wqk_tiles,
            "wv": wv,
            "wo": wo,
            "bo": bo,
            "mcol": np.ascontiguousarray(m[b].reshape(NT, P).T),
            "tri": tri,
            "ones": ones,
            "sel2": sel2,
        })

    trace = bool(int(os.environ.get("TRN_TRACE", "0")))
    res = run_bass_kernel_spmd(
        nc, in_maps, core_ids=list(range(8)), trace=trace,
    )
    LAST_RESULTS = res
    out = np.stack([res.results[b]["out"] for b in range(8)], axis=0)
    return out.astype(np.float32)
